# revision 9
# baseline (speedup 1.0000x reference)
import numpy as np
import ml_dtypes
from concurrent.futures import ThreadPoolExecutor

import concourse.bass as bass
import concourse.bacc as bacc
import concourse.mybir as mybir
import concourse.tile as tile
from concourse.bass import broadcast_tensor_aps
from concourse import bass_utils

B, T, N, F = 32, 4096, 11, 16
H = 2 * F                 # 32
NF = N * F                # 176
MH = N * H                # 352
MF = N * F                # 176
LN_EPS = 1e-5
NCORES = 8
BPC = B // NCORES         # 4 batches per core
NCHUNK = 1                # single call
BPCALL = BPC // NCHUNK    # batches per core per call
TT = 128                  # t per tile
GG = 8                    # tiles per DMA slab
TS = TT * GG              # 1024 t per slab
NSLABS = T // TS          # 4
K1B = NF - 128            # 48 data rows in second MM1 chunk
BF = mybir.dt.bfloat16
F32 = mybir.dt.float32
F16 = mybir.dt.float16
I8 = mybir.dt.int8
U8 = mybir.dt.uint8
XQS = 126.5               # x quant scale divisor (margin for bf16 round)

_CACHE = {}


def _build_program():
    nc = bacc.Bacc("TRN2", target_bir_lowering=False, debug=False,
                   num_devices=NCORES)
    x_d = nc.dram_tensor("x", [BPCALL, T, NF], U8, kind="ExternalInput").ap()
    xs_d = nc.dram_tensor("xs", [BPCALL, T, N], BF,
                          kind="ExternalInput").ap()
    c_d = nc.dram_tensor("cw", [BPCALL, 177, MH], BF,
                         kind="ExternalInput").ap()
    d_d = nc.dram_tensor("dw", [128, 704], BF, kind="ExternalInput").ap()
    o_d = nc.dram_tensor("ones1", [1, 128], BF, kind="ExternalInput").ap()
    i_d = nc.dram_tensor("ident", [128, 128], BF, kind="ExternalInput").ap()
    g_d = nc.dram_tensor("gb", [128, 3], F32, kind="ExternalInput").ap()
    yq_d = nc.dram_tensor("yq", [BPCALL, T, MF], I8,
                          kind="ExternalOutput").ap()
    ys_d = nc.dram_tensor("ys", [BPCALL, T, N], F16,
                          kind="ExternalOutput").ap()

    with tile.TileContext(nc) as tc:
        with (
            tc.tile_pool(name="wpool", bufs=1) as wpool,
            tc.tile_pool(name="xin", bufs=3) as xin_pool,
            tc.tile_pool(name="yout", bufs=3) as yout_pool,
            tc.tile_pool(name="ps_xt", bufs=2, space="PSUM") as ps_xt,
            tc.tile_pool(name="ps_hc", bufs=2, space="PSUM") as ps_hc,
            tc.tile_pool(name="ps_ut", bufs=2, space="PSUM") as ps_ut,
            tc.tile_pool(name="ps_o", bufs=2, space="PSUM") as ps_o,
            tc.tile_pool(name="work", bufs=3) as work,
        ):
            ident = wpool.tile([128, 128], BF, tag="ident")
            nc.sync.dma_start(ident[:, :], i_d[:, :])
            d_sb = wpool.tile([128, 704], BF, tag="dw")
            nc.sync.dma_start(d_sb[:, :], d_d[:, :])
            ones_sb = wpool.tile([1, 128], BF, tag="ones1")
            nc.sync.dma_start(ones_sb[:, :], o_d[:, :])
            gb = wpool.tile([128, 3], F32, tag="gb")
            nc.sync.dma_start(gb[:, :], g_d[:, :])
            c_sb = []
            for b in range(BPCALL):
                cb = wpool.tile([128, 1056], BF, tag=f"cw{b}")
                nc.sync.dma_start(cb[:, 0:MH], c_d[b, 0:128, :])
                nc.sync.dma_start(cb[0:K1B, MH:2 * MH], c_d[b, 128:NF, :])
                nc.sync.dma_start(cb[0:1, 2 * MH:3 * MH], c_d[b, NF:NF + 1, :])
                c_sb.append(cb)

            for b in range(BPCALL):
                for s in range(NSLABS):
                    t0 = s * TS
                    xq_slab = xin_pool.tile([TT, GG * NF], U8, tag="xq_slab")
                    xv = x_d[b, t0:t0 + TS, :].rearrange(
                        "(g p) f -> p g f", p=TT)
                    nc.sync.dma_start(
                        xq_slab[:, :].rearrange("p (g f) -> p g f", g=GG), xv)
                    xs_slab = xin_pool.tile([TT, GG * N], BF, tag="xs_slab")
                    xsv = xs_d[b, t0:t0 + TS, :].rearrange(
                        "(g p) n -> p g n", p=TT)
                    nc.sync.dma_start(
                        xs_slab[:, :].rearrange("p (g n) -> p g n", g=GG), xsv)
                    # dequant: x = (u - 127) * scale, per (t,n) group of F
                    xm_slab = xin_pool.tile([TT, GG * NF], BF, tag="xm_slab")
                    nc.scalar.activation(
                        xm_slab[:, :], xq_slab[:, :],
                        mybir.ActivationFunctionType.Copy, bias=-127.0)
                    x_slab = xin_pool.tile([TT, GG * NF], BF, tag="x_slab")
                    xd_v = x_slab[:, :].rearrange("p (a f) -> p a f", f=F)
                    xm_v = xm_slab[:, :].rearrange("p (a f) -> p a f", f=F)
                    xs_v = xs_slab[:, :].rearrange("p (a o) -> p a o", o=1)
                    xd_b, xsb_b = broadcast_tensor_aps(xd_v, xs_v)
                    nc.vector.tensor_mul(xd_b, xm_v, xsb_b)
                    q_slab = yout_pool.tile([TT, GG * MF], I8, tag="q_slab")
                    s_slab = yout_pool.tile([TT, GG * N], F16, tag="s_slab")
                    for g in range(GG):
                        xg = x_slab[:, g * NF:(g + 1) * NF]
                        # ---- transpose x tile to [(n,f), t]
                        xt_ps = ps_xt.tile([128, 256], BF, tag="xt_ps")
                        nc.tensor.transpose(xt_ps[:, 0:128], xg[:, 0:128],
                                            ident[:, :])
                        nc.tensor.transpose(xt_ps[0:48, 128:256],
                                            xg[:, 128:176], ident[:, :])
                        xt_sb = work.tile([128, 256], BF, tag="xt_sb")
                        nc.scalar.copy(xt_sb[:, :], xt_ps[:, :])
                        # ---- MM1: hc[t,(m,h')] centered (mean folded into C)
                        hc_ps = ps_hc.tile([128, MH], F32, tag="hc_ps")
                        nc.tensor.matmul(hc_ps[:, :], xt_sb[:, 0:128],
                                         c_sb[b][:, 0:MH],
                                         start=True, stop=False)
                        nc.tensor.matmul(hc_ps[:, :],
                                         xt_sb[0:K1B, 128:256],
                                         c_sb[b][0:K1B, MH:2 * MH],
                                         start=False, stop=False)
                        nc.tensor.matmul(hc_ps[:, :], ones_sb[0:1, :],
                                         c_sb[b][0:1, 704:1056],
                                         start=False, stop=True)
                        # ---- variance: sum of squares over h' groups
                        h2 = work.tile([128, MH], F32, tag="h2")
                        nc.scalar.square(h2[:, :], hc_ps[:, :])
                        v2 = work.tile([128, N], F32, tag="v2")
                        nc.vector.reduce_sum(
                            v2[:, :],
                            h2[:, :].rearrange("p (m h) -> p m h", h=H),
                            axis=mybir.AxisListType.X)
                        sd = work.tile([128, N], F32, tag="sd")
                        nc.scalar.activation(
                            sd[:, :], v2[:, :],
                            mybir.ActivationFunctionType.Sqrt,
                            bias=gb[:, 2:3], scale=1.0 / H)
                        rs = work.tile([128, N], F32, tag="rs")
                        nc.vector.reciprocal(rs[:, :], sd[:, :])
                        # ---- u = hc * rs  (broadcast rs over h')
                        u_sb = work.tile([128, MH], BF, tag="u_sb")
                        u_v = u_sb[:, :].rearrange("p (m h) -> p m h", h=H)
                        hc_v = hc_ps[:, :].rearrange("p (m h) -> p m h", h=H)
                        rs_v = rs[:, :].rearrange("p (m o) -> p m o", o=1)
                        u_b, rs_b = broadcast_tensor_aps(u_v, rs_v)
                        nc.vector.tensor_mul(u_b, hc_v, rs_b)
                        # ---- transpose u to [(m,h'), t] in 3 chunks
                        ut_ps = ps_ut.tile([128, 384], BF, tag="ut_ps")
                        nc.tensor.transpose(ut_ps[:, 0:128], u_sb[:, 0:128],
                                            ident[:, :])
                        nc.tensor.transpose(ut_ps[:, 128:256],
                                            u_sb[:, 128:256], ident[:, :])
                        nc.tensor.transpose(ut_ps[0:96, 256:384],
                                            u_sb[:, 256:352], ident[:, :])
                        # ---- gelu(u*gamma+beta): gamma/beta per-partition
                        hgt = work.tile([128, 384], BF, tag="hgt")
                        nc.scalar.activation(
                            hgt[:, :], ut_ps[:, :],
                            mybir.ActivationFunctionType.Gelu,
                            bias=gb[:, 1:2], scale=gb[:, 0:1])
                        # ---- MM2: out2[t,(m,f)] = hgT.T @ D (+b2 row)
                        o_ps = ps_o.tile([128, MF], F32, tag="o_ps")
                        nc.tensor.matmul(o_ps[:, :], hgt[:, 0:128],
                                         d_sb[:, 0:176],
                                         start=True, stop=False)
                        nc.tensor.matmul(o_ps[:, :], hgt[:, 128:256],
                                         d_sb[:, 176:352],
                                         start=False, stop=False)
                        nc.tensor.matmul(o_ps[:, :], hgt[0:96, 256:384],
                                         d_sb[0:96, 352:528],
                                         start=False, stop=False)
                        nc.tensor.matmul(o_ps[:, :], ones_sb[0:1, :],
                                         d_sb[0:1, 528:704],
                                         start=False, stop=True)
                        # ---- int8 quantize per (t, m) group of F values.
                        # scale stored as s/127 in f16; quantization uses the
                        # reciprocal of the STORED value so encode == decode.
                        sm = work.tile([128, N], F32, tag="sm")
                        nc.vector.tensor_reduce(
                            sm[:, :],
                            o_ps[:, :].rearrange("p (m f) -> p m f", f=F),
                            axis=mybir.AxisListType.X,
                            op=mybir.AluOpType.max,
                            apply_absolute_value=True)
                        ss = s_slab[:, g * N:(g + 1) * N]
                        nc.vector.tensor_scalar(
                            ss, sm[:, :], 1e-30, 1.0 / 127.0,
                            op0=mybir.AluOpType.max,
                            op1=mybir.AluOpType.mult)
                        iv = work.tile([128, N], F32, tag="iv")
                        nc.vector.reciprocal(iv[:, :], ss)
                        qf = work.tile([128, MF], F32, tag="qf")
                        qf_v = qf[:, :].rearrange("p (m f) -> p m f", f=F)
                        o_v = o_ps[:, :].rearrange("p (m f) -> p m f", f=F)
                        iv_v = iv[:, :].rearrange("p (m o) -> p m o", o=1)
                        qf_b, iv_b = broadcast_tensor_aps(qf_v, iv_v)
                        nc.vector.tensor_mul(qf_b, o_v, iv_b)
                        nc.scalar.copy(q_slab[:, g * MF:(g + 1) * MF],
                                       qf[:, :])
                    qv = yq_d[b, t0:t0 + TS, :].rearrange(
                        "(g p) f -> p g f", p=TT)
                    nc.sync.dma_start(
                        qv, q_slab[:, :].rearrange("p (g f) -> p g f", g=GG))
                    sv = ys_d[b, t0:t0 + TS, :].rearrange(
                        "(g p) n -> p g n", p=TT)
                    nc.sync.dma_start(
                        sv, s_slab[:, :].rearrange("p (g n) -> p g n", g=GG))
    nc.compile()
    return nc


def _prep(x, lab_idx, projection, bias, w1, b1, ln_g, ln_b, w2, b2):
    """Returns per-chunk in_maps: chunks[k][core] covers global batch
    core*BPC + k*BPCALL ... + BPCALL."""
    f32 = np.float32
    bf = ml_dtypes.bfloat16
    x = np.asarray(x, f32)
    lab = np.asarray(lab_idx).astype(np.int64)
    W = np.asarray(projection, f32)[lab]            # [B,11,11]
    Bb = np.asarray(bias, f32)[lab][:, 0]           # [B,11,16]
    w1 = np.asarray(w1, f32); b1 = np.asarray(b1, f32)
    ln_g = np.asarray(ln_g, f32); ln_b = np.asarray(ln_b, f32)
    w2 = np.asarray(w2, f32); b2 = np.asarray(b2, f32)

    w1c = w1 - w1.mean(axis=1, keepdims=True)       # [16,32]
    C = np.einsum('bnm,fh->bnfmh', W, w1c).reshape(B, NF, MH)
    biasc = (b1 - b1.mean())[None, None, :] + Bb @ w1c     # [B,11,32]
    Cpack = np.concatenate(
        [C, biasc.reshape(B, 1, MH)], axis=1).astype(bf)   # [B,177,352]

    D = np.zeros((352, 176), f32)
    for m in range(N):
        D[m * H:(m + 1) * H, m * F:(m + 1) * F] = w2
    Dpack = np.zeros((128, 704), f32)
    Dpack[:, 0:176] = D[0:128]
    Dpack[:, 176:352] = D[128:256]
    Dpack[0:96, 352:528] = D[256:352]
    Dpack[0, 528:704] = np.tile(b2, N)
    Dbf = Dpack.astype(bf)

    gb = np.zeros((128, 3), f32)
    gb[:, 2] = LN_EPS
    gb[:, 0] = np.tile(ln_g, 128 // H)
    gb[:, 1] = np.tile(ln_b, 128 // H)
    ident = np.eye(128, dtype=bf)
    ones1 = np.ones((1, 128), bf)

    # int8-quantize x per (t, n) group of F values; scale stored bf16 so the
    # device dequant (u - 127) * scale reproduces the host decode exactly.
    xr = x.reshape(-1, F)
    a = np.abs(xr)
    m = np.maximum(a[:, :8], a[:, 8:])
    m = np.maximum(m[:, :4], m[:, 4:])
    m = np.maximum(m[:, :2], m[:, 2:])
    m = np.maximum(m[:, 0], m[:, 1])
    sc = (np.maximum(m, 1e-30) * (1.0 / XQS)).astype(bf)
    inv = 1.0 / sc.astype(f32)
    tmp = xr * inv[:, None]
    np.add(tmp, 127.5, out=tmp)
    xq = tmp.astype(np.uint8).reshape(B, T, NF)
    xs = sc.reshape(B, T, N)

    chunks = []
    for k in range(NCHUNK):
        in_maps = []
        for i in range(NCORES):
            b0 = i * BPC + k * BPCALL
            sl = slice(b0, b0 + BPCALL)
            in_maps.append({
                "x": xq[sl],
                "xs": xs[sl],
                "cw": Cpack[sl],
                "dw": Dbf,
                "ident": ident,
                "ones1": ones1,
                "gb": gb,
            })
        chunks.append(in_maps)
    return chunks


def _warm(nc):
    """One sequential call with all-zero inputs: warms the jit/NEFF caches
    so the pipelined threads never race a first compile. Zero buffers
    upload fast."""
    bf = ml_dtypes.bfloat16
    zmaps = [{
        "x": np.zeros((BPCALL, T, NF), np.uint8),
        "xs": np.zeros((BPCALL, T, N), bf),
        "cw": np.zeros((BPCALL, 177, MH), bf),
        "dw": np.zeros((128, 704), bf),
        "ident": np.zeros((128, 128), bf),
        "ones1": np.zeros((1, 128), bf),
        "gb": np.zeros((128, 3), np.float32),
    } for _ in range(NCORES)]
    bass_utils.run_bass_kernel_spmd(nc, zmaps, core_ids=list(range(NCORES)))


def kernel(**inputs):
    if "nc" not in _CACHE:
        _CACHE["nc"] = _build_program()
        _warm(_CACHE["nc"])
    nc = _CACHE["nc"]
    chunks = _prep(**inputs)

    def run_chunk(k):
        return bass_utils.run_bass_kernel_spmd(nc, chunks[k],
                                               core_ids=list(range(NCORES)))

    with ThreadPoolExecutor(max_workers=NCHUNK) as ex:
        futs = [ex.submit(run_chunk, k) for k in range(NCHUNK)]
        results = [f.result() for f in futs]

    y = np.empty((B, T, N, F), np.float32)
    for k in range(NCHUNK):
        res = results[k]
        for i in range(NCORES):
            b0 = i * BPC + k * BPCALL
            yq = np.asarray(res.results[i]["yq"])    # [BPCALL, T, MF] int8
            ys = np.asarray(res.results[i]["ys"])    # [BPCALL, T, N] f16
            yb = yq.reshape(BPCALL, T, N, F).astype(np.float32)
            yb *= ys.astype(np.float32)[:, :, :, None]
            y[b0:b0 + BPCALL] = yb
    return y
